# revision 19
# baseline (speedup 1.0000x reference)
"""Trainium2 Bass kernel for a GCN message-passing layer (v2, f16 stream).

Reference computation (per node i):
    out[i] = sum_j edges[i,j] * (w1 @ concat(x[j], dist[i,j])) + w2 @ x[i]
which factors into:
    xmsg = x @ w1x.T                       (w1x = w1[:, :128])
    agg  = edges @ xmsg                    (big GEMM, contraction over j)
    dw   = einsum('ij,ijc->ic', edges, dist)
    out  = agg + dw @ w1d.T + x @ w2.T     (w1d = w1[:, 128:130])

Sharding: rows i (targets) split across 8 NeuronCores; x/w1/w2 replicated.

The kernel is HBM-bound: each core must read its edges slice and both
distance channels once.  v2 streams all three as float16 (48 MB/core vs
96 MB in f32; quantization error ~4e-4 rel L2, far under the 2e-2 gate)
and uploads them pre-transposed so the PE does no on-chip transposes:

  e3/da3/db3: [j, i] granule-major layout, so every DMA descriptor is a
  contiguous multi-KB run per partition and every matmul rhs tile is
  [j-part, i-free] as the PE wants it.

Dist term without a serialized epilogue: the DVE forms p_c = E^T .* D_c^T
per granule (f16), and a rank-1 stationary W_c[j,f] = w1d[f,c] (constant
across j) turns sum_j p_c[j,i] * w1d[f,c] into a regular accumulating
matmul into the same PSUM banks as the main GEMM:
    out^T[f,i] += sum_j W_c[j,f] * p_c[j,i].

Rings: sync HWDGE carries E + D-ch0, scalar HWDGE carries D-ch1, gpsimd
carries the small prologue (weights, x^T pieces) and the output stores.
The last granule is split into single-chunk pieces so the kernel tail
(compute on last-arriving data) is short.
"""

import os

import numpy as np

import concourse.bacc as bacc
import concourse.mybir as mybir
from concourse.tile import TileContext

F32 = mybir.dt.float32
F16 = mybir.dt.float16
F8 = mybir.dt.float8e4
P = 128

# problem dims (hardcoded per contract)
N_FULL = 8192
F_IN = 128
F_OUT = 128
N_CORES = 8
KB = 2  # j-chunks (of 128) per streamed granule

LAST_RESULT = None  # BassKernelResults of the most recent kernel() call


def _sched(nch, kb):
    """Granule schedule: (granule, chunk-offset, n-chunks) tuples.

    The final granule is split into single chunks so the tail compute
    pipeline starts on partial data instead of waiting for the full
    granule.
    """
    ngr = nch // kb
    s = [(g, 0, kb) for g in range(ngr - 2)]
    for g in (ngr - 2, ngr - 1):
        s += [(g, b, 1) for b in range(kb)]
    return s


def build(n=N_FULL, rows=N_FULL // N_CORES, kb=KB):
    f = F_IN
    assert n % P == 0 and rows % 2 == 0
    nch = n // P
    assert nch % kb == 0
    ngr = nch // kb
    h = rows // 2  # output free-dim half, one PSUM bank each
    assert h <= 512
    pw = min(1024, n)  # xT prologue piece width
    npc = n // pw

    nc = bacc.Bacc()
    e3_d = nc.declare_dram_parameter("e3", [ngr, P, kb, rows], F16, isOutput=False)
    # distance channels stream as fp8e4 (8 MB each per core instead of 16);
    # quantization error on the dist term is ~0.5% of its share -> ~4e-3
    # rel L2 overall, still 5x under the gate
    da_d = nc.declare_dram_parameter("da3", [ngr, P, kb, rows], F8, isOutput=False)
    db_d = nc.declare_dram_parameter("db3", [ngr, P, kb, rows], F8, isOutput=False)
    xT_d = nc.declare_dram_parameter("xT", [f, n], F16, isOutput=False)
    xTs_d = nc.declare_dram_parameter("xT_self", [f, rows], F16, isOutput=False)
    w1xT_d = nc.declare_dram_parameter("w1xT", [f, F_OUT], F16, isOutput=False)
    w2T_d = nc.declare_dram_parameter("w2T", [f, F_OUT], F16, isOutput=False)
    wd0_d = nc.declare_dram_parameter("wd0", [P, F_OUT], F16, isOutput=False)
    wd1_d = nc.declare_dram_parameter("wd1", [P, F_OUT], F16, isOutput=False)
    o_d = nc.declare_dram_parameter("outT", [F_OUT, rows], F32, isOutput=True)

    sched = _sched(nch, kb)

    with TileContext(nc) as tc:
        with (
            tc.tile_pool(name="const", bufs=1) as cpool,
            tc.tile_pool(name="stream", bufs=2) as pool,
            tc.tile_pool(name="psum", bufs=1, space="PSUM") as pp,
        ):
            def load_granule(gi, g, b0, nb):
                et = pool.tile([P, nb, rows], F16, tag="E", bufs=4, name=f"et{gi}")
                nc.sync.dma_start(et, e3_d[g, :, b0 : b0 + nb, :])
                da = pool.tile([P, nb, rows], F8, tag="DA8", bufs=4, name=f"da{gi}")
                nc.sync.dma_start(da, da_d[g, :, b0 : b0 + nb, :])
                db = pool.tile([P, nb, rows], F8, tag="DB8", bufs=4, name=f"db{gi}")
                nc.scalar.dma_start(db, db_d[g, :, b0 : b0 + nb, :])
                return et, da, db

            # the big streams start at t=0: preload the first three granules
            pre = {}
            for gi in (0, 1, 2):
                pre[gi] = load_granule(gi, *sched[gi])

            # ---------------- prologue (gpsimd ring) ----------------
            w1xT = cpool.tile([f, F_OUT], F16)
            nc.gpsimd.dma_start(w1xT, w1xT_d[:, :])
            w2T = cpool.tile([f, F_OUT], F16)
            nc.gpsimd.dma_start(w2T, w2T_d[:, :])
            wd0 = cpool.tile([P, F_OUT], F16)
            nc.gpsimd.dma_start(wd0, wd0_d[:, :])
            wd1 = cpool.tile([P, F_OUT], F16)
            nc.gpsimd.dma_start(wd1, wd1_d[:, :])
            xTs_sb = cpool.tile([f, rows], F16)
            nc.gpsimd.dma_start(xTs_sb, xTs_d[:, :])
            xTp = []
            for b in range(npc):
                t = cpool.tile([f, pw], F16, name=f"xTp{b}")
                nc.gpsimd.dma_start(t, xT_d[:, b * pw : (b + 1) * pw])
                xTp.append(t)

            # xmsg[j, f] chunks land here (f16, stationary for main GEMM)
            xmsg = cpool.tile([P, nch, f], F16)

            def stage_xmsg(g):
                # one PSUM bank stages the kb chunks of granule g
                xm = pp.tile([P, kb * f], F32, tag="xstage", bufs=2, name=f"xm{g}")
                for r in range(kb):
                    ch = kb * g + r
                    b, off = divmod(ch * P, pw)
                    nc.tensor.matmul(
                        xm[:, r * f : (r + 1) * f],
                        xTp[b][:, off : off + P],
                        w1xT,
                        start=True,
                        stop=True,
                    )
                nc.scalar.copy(xmsg[:, kb * g : kb * (g + 1)], xm)

            # output accumulators: one PSUM bank per output half
            agg0 = pp.tile([P, h], F32, tag="agg0")
            agg1 = pp.tile([P, h], F32, tag="agg1")
            # self-connection term starts the accumulation
            nc.tensor.matmul(agg0, w2T, xTs_sb[:, 0:h], start=True, stop=False)
            nc.tensor.matmul(agg1, w2T, xTs_sb[:, h : 2 * h], start=True, stop=False)

            # ---------------- main loop ----------------
            staged = set()
            last_i = len(sched) - 1
            for gi, (g, b0, nb) in enumerate(sched):
                if g not in staged:
                    stage_xmsg(g)
                    staged.add(g)

                if gi in pre:
                    et, da, db = pre.pop(gi)
                else:
                    et, da, db = load_granule(gi, g, b0, nb)

                # upcast fp8 dist tiles to f16.  GPSIMD is useless here
                # (measured ~6x slower than DVE), so balance DVE vs scalar:
                # DVE has the 74us of products, scalar has ~79us of casts and
                # copies, so channel A's casts alternate DVE/scalar to land
                # both engines near ~99us
                daf = pool.tile([P, nb, rows], F16, tag="DAF", bufs=3, name=f"daf{gi}")
                if gi % 3 == 2:
                    nc.scalar.copy(daf, da)
                else:
                    nc.vector.tensor_copy(daf, da)
                dbf = pool.tile([P, nb, rows], F16, tag="DBF", bufs=3, name=f"dbf{gi}")
                nc.scalar.copy(dbf, db)

                # dist products on the DVE via the true TENSOR_TENSOR opcode:
                # unlike TensorScalarPtr (1x only), it has the 2x_1p uop for
                # 16-bit step-1 operands -> ~34us per channel per core
                pa = pool.tile([P, nb, rows], F16, tag="PA", bufs=3, name=f"pa{gi}")
                nc.vector.tensor_tensor(pa, et, daf, mybir.AluOpType.mult)
                pb = pool.tile([P, nb, rows], F16, tag="PB", bufs=3, name=f"pb{gi}")
                nc.vector.tensor_tensor(pb, et, dbf, mybir.AluOpType.mult)

                if gi != last_i:
                    # main GEMM chunks
                    for b in range(nb):
                        ch = g * kb + b0 + b
                        nc.tensor.matmul(
                            agg0, xmsg[:, ch], et[:, b, 0:h], start=False, stop=False
                        )
                        nc.tensor.matmul(
                            agg1, xmsg[:, ch], et[:, b, h : 2 * h], start=False, stop=False
                        )
                    # dist-term chunks, grouped per stationary weight
                    for b in range(nb):
                        nc.tensor.matmul(
                            agg0, wd0, pa[:, b, 0:h], start=False, stop=False
                        )
                        nc.tensor.matmul(
                            agg1, wd0, pa[:, b, h : 2 * h], start=False, stop=False
                        )
                    for b in range(nb):
                        nc.tensor.matmul(
                            agg0, wd1, pb[:, b, 0:h], start=False, stop=False
                        )
                        nc.tensor.matmul(
                            agg1, wd1, pb[:, b, h : 2 * h], start=False, stop=False
                        )
                else:
                    # tail: finish bank 0 entirely first so its copy+store
                    # overlaps bank 1's final matmuls
                    ch = g * kb + b0
                    nc.tensor.matmul(agg0, xmsg[:, ch], et[:, 0, 0:h], start=False, stop=False)
                    nc.tensor.matmul(agg0, wd0, pa[:, 0, 0:h], start=False, stop=False)
                    nc.tensor.matmul(agg0, wd1, pb[:, 0, 0:h], start=False, stop=True)
                    out0 = pool.tile([P, h], F32, tag="osb0")
                    nc.scalar.copy(out0, agg0)
                    # tail stores ride the sync ring: its load triggers are
                    # all done by now, and gpsimd is busy with products
                    nc.sync.dma_start(o_d[:, 0:h], out0)

                    nc.tensor.matmul(agg1, xmsg[:, ch], et[:, 0, h : 2 * h], start=False, stop=False)
                    nc.tensor.matmul(agg1, wd0, pa[:, 0, h : 2 * h], start=False, stop=False)
                    nc.tensor.matmul(agg1, wd1, pb[:, 0, h : 2 * h], start=False, stop=True)
                    out1 = pool.tile([P, h], F32, tag="osb1")
                    nc.scalar.copy(out1, agg1)
                    nc.sync.dma_start(o_d[:, h : 2 * h], out1)

    nc.compile()
    return nc


def _prep_in_maps(inputs, rows, n_cores, kb):
    import ml_dtypes

    f16 = np.float16
    f8 = ml_dtypes.float8_e4m3
    x = np.asarray(inputs["x"], np.float32)
    edges = np.asarray(inputs["edges"], np.float32)
    dist = np.asarray(inputs["distance_matrix"], np.float32)
    w1 = np.asarray(inputs["w1"], np.float32)
    w2 = np.asarray(inputs["w2"], np.float32)
    f = x.shape[1]
    n = edges.shape[1]
    nch = n // P
    ngr = nch // kb

    xT16 = np.ascontiguousarray(x.T.astype(f16))  # [f, n]
    w1xT = np.ascontiguousarray(w1[:, :f].T.astype(f16))
    w2T = np.ascontiguousarray(w2.T.astype(f16))
    w1d = w1[:, f:].astype(f16)  # [F, 2]
    wd0 = np.ascontiguousarray(np.broadcast_to(w1d[:, 0][None, :], (P, f)))
    wd1 = np.ascontiguousarray(np.broadcast_to(w1d[:, 1][None, :], (P, f)))

    def g3(mat, dt):  # [rows, n] f32 -> [ngr, 128, kb, rows], j-major granules
        t = mat.T.astype(dt)  # [n, rows]
        return np.ascontiguousarray(
            t.reshape(ngr, kb, P, rows).transpose(0, 2, 1, 3)
        )

    in_maps = []
    for c in range(n_cores):
        i0, i1 = c * rows, (c + 1) * rows
        in_maps.append(
            {
                "e3": g3(edges[i0:i1], f16),
                "da3": g3(dist[i0:i1, :, 0], f8),
                "db3": g3(dist[i0:i1, :, 1], f8),
                "xT": xT16,
                "xT_self": np.ascontiguousarray(xT16[:, i0:i1]),
                "w1xT": w1xT,
                "w2T": w2T,
                "wd0": wd0,
                "wd1": wd1,
            }
        )
    return in_maps


def _run(inputs, n, rows_per_core, n_cores, kb, trace=False):
    from concourse.bass_utils import run_bass_kernel_spmd

    in_maps = _prep_in_maps(inputs, rows_per_core, n_cores, kb)
    nc = build(n=n, rows=rows_per_core, kb=kb)
    res = run_bass_kernel_spmd(nc, in_maps, core_ids=list(range(n_cores)), trace=trace)

    global LAST_RESULT
    LAST_RESULT = res

    out = np.concatenate([r["outT"].T for r in res.results], axis=0)
    return out


def kernel(**inputs) -> np.ndarray:
    trace = os.environ.get("KERNEL_TRACE", "0") == "1"
    return _run(
        inputs,
        n=N_FULL,
        rows_per_core=N_FULL // N_CORES,
        n_cores=N_CORES,
        kb=KB,
        trace=trace,
    )


# revision 20
# speedup vs baseline: 1.1596x; 1.1596x over previous
"""Trainium2 Bass kernel for a GCN message-passing layer (v2, f16 stream).

Reference computation (per node i):
    out[i] = sum_j edges[i,j] * (w1 @ concat(x[j], dist[i,j])) + w2 @ x[i]
which factors into:
    xmsg = x @ w1x.T                       (w1x = w1[:, :128])
    agg  = edges @ xmsg                    (big GEMM, contraction over j)
    dw   = einsum('ij,ijc->ic', edges, dist)
    out  = agg + dw @ w1d.T + x @ w2.T     (w1d = w1[:, 128:130])

Sharding: rows i (targets) split across 8 NeuronCores; x/w1/w2 replicated.

The kernel is HBM-bound: each core must read its edges slice and both
distance channels once.  v2 streams all three as float16 (48 MB/core vs
96 MB in f32; quantization error ~4e-4 rel L2, far under the 2e-2 gate)
and uploads them pre-transposed so the PE does no on-chip transposes:

  e3/da3/db3: [j, i] granule-major layout, so every DMA descriptor is a
  contiguous multi-KB run per partition and every matmul rhs tile is
  [j-part, i-free] as the PE wants it.

Dist term without a serialized epilogue: the DVE forms p_c = E^T .* D_c^T
per granule (f16), and a rank-1 stationary W_c[j,f] = w1d[f,c] (constant
across j) turns sum_j p_c[j,i] * w1d[f,c] into a regular accumulating
matmul into the same PSUM banks as the main GEMM:
    out^T[f,i] += sum_j W_c[j,f] * p_c[j,i].

Rings: sync HWDGE carries E + D-ch0, scalar HWDGE carries D-ch1, gpsimd
carries the small prologue (weights, x^T pieces) and the output stores.
The last granule is split into single-chunk pieces so the kernel tail
(compute on last-arriving data) is short.
"""

import os

import numpy as np

import concourse.bacc as bacc
import concourse.mybir as mybir
from concourse.tile import TileContext

F32 = mybir.dt.float32
F16 = mybir.dt.float16
F8 = mybir.dt.float8e4
P = 128

# problem dims (hardcoded per contract)
N_FULL = 8192
F_IN = 128
F_OUT = 128
N_CORES = 8
KB = 4  # j-chunks (of 128) per streamed granule

LAST_RESULT = None  # BassKernelResults of the most recent kernel() call


def _sched(nch, kb):
    """Granule schedule: (granule, chunk-offset, n-chunks) tuples.

    The final granule is split into single chunks so the tail compute
    pipeline starts on partial data instead of waiting for the full
    granule.
    """
    ngr = nch // kb
    s = [(g, 0, kb) for g in range(ngr - 1)]
    s += [(ngr - 1, b, 1) for b in range(kb)]
    return s


def build(n=N_FULL, rows=N_FULL // N_CORES, kb=KB):
    f = F_IN
    assert n % P == 0 and rows % 2 == 0
    nch = n // P
    assert nch % kb == 0
    ngr = nch // kb
    h = rows // 2  # output free-dim half, one PSUM bank each
    assert h <= 512
    pw = min(1024, n)  # xT prologue piece width
    npc = n // pw

    nc = bacc.Bacc()
    e3_d = nc.declare_dram_parameter("e3", [ngr, P, kb, rows], F16, isOutput=False)
    # distance channels stream as fp8e4 (8 MB each per core instead of 16);
    # quantization error on the dist term is ~0.5% of its share -> ~4e-3
    # rel L2 overall, still 5x under the gate
    da_d = nc.declare_dram_parameter("da3", [ngr, P, kb, rows], F8, isOutput=False)
    db_d = nc.declare_dram_parameter("db3", [ngr, P, kb, rows], F8, isOutput=False)
    xT_d = nc.declare_dram_parameter("xT", [f, n], F16, isOutput=False)
    xTs_d = nc.declare_dram_parameter("xT_self", [f, rows], F16, isOutput=False)
    w1xT_d = nc.declare_dram_parameter("w1xT", [f, F_OUT], F16, isOutput=False)
    w2T_d = nc.declare_dram_parameter("w2T", [f, F_OUT], F16, isOutput=False)
    wd0_d = nc.declare_dram_parameter("wd0", [P, F_OUT], F16, isOutput=False)
    wd1_d = nc.declare_dram_parameter("wd1", [P, F_OUT], F16, isOutput=False)
    o_d = nc.declare_dram_parameter("outT", [F_OUT, rows], F32, isOutput=True)

    sched = _sched(nch, kb)

    with TileContext(nc) as tc:
        with (
            tc.tile_pool(name="const", bufs=1) as cpool,
            tc.tile_pool(name="stream", bufs=2) as pool,
            tc.tile_pool(name="psum", bufs=1, space="PSUM") as pp,
        ):
            def load_granule(gi, g, b0, nb):
                et = pool.tile([P, nb, rows], F16, tag="E", bufs=4, name=f"et{gi}")
                nc.sync.dma_start(et, e3_d[g, :, b0 : b0 + nb, :])
                da = pool.tile([P, nb, rows], F8, tag="DA8", bufs=4, name=f"da{gi}")
                nc.sync.dma_start(da, da_d[g, :, b0 : b0 + nb, :])
                db = pool.tile([P, nb, rows], F8, tag="DB8", bufs=4, name=f"db{gi}")
                nc.scalar.dma_start(db, db_d[g, :, b0 : b0 + nb, :])
                return et, da, db

            # the big streams start at t=0: preload the first three granules
            pre = {}
            for gi in (0, 1, 2):
                pre[gi] = load_granule(gi, *sched[gi])

            # ---------------- prologue (gpsimd ring) ----------------
            w1xT = cpool.tile([f, F_OUT], F16)
            nc.gpsimd.dma_start(w1xT, w1xT_d[:, :])
            w2T = cpool.tile([f, F_OUT], F16)
            nc.gpsimd.dma_start(w2T, w2T_d[:, :])
            wd0 = cpool.tile([P, F_OUT], F16)
            nc.gpsimd.dma_start(wd0, wd0_d[:, :])
            wd1 = cpool.tile([P, F_OUT], F16)
            nc.gpsimd.dma_start(wd1, wd1_d[:, :])
            xTs_sb = cpool.tile([f, rows], F16)
            nc.gpsimd.dma_start(xTs_sb, xTs_d[:, :])
            xTp = []
            for b in range(npc):
                t = cpool.tile([f, pw], F16, name=f"xTp{b}")
                nc.gpsimd.dma_start(t, xT_d[:, b * pw : (b + 1) * pw])
                xTp.append(t)

            # xmsg[j, f] chunks land here (f16, stationary for main GEMM)
            xmsg = cpool.tile([P, nch, f], F16)

            def stage_xmsg(g):
                # one PSUM bank stages the kb chunks of granule g
                xm = pp.tile([P, kb * f], F32, tag="xstage", bufs=2, name=f"xm{g}")
                for r in range(kb):
                    ch = kb * g + r
                    b, off = divmod(ch * P, pw)
                    nc.tensor.matmul(
                        xm[:, r * f : (r + 1) * f],
                        xTp[b][:, off : off + P],
                        w1xT,
                        start=True,
                        stop=True,
                    )
                nc.scalar.copy(xmsg[:, kb * g : kb * (g + 1)], xm)

            # output accumulators: one PSUM bank per output half
            agg0 = pp.tile([P, h], F32, tag="agg0")
            agg1 = pp.tile([P, h], F32, tag="agg1")
            # self-connection term starts the accumulation
            nc.tensor.matmul(agg0, w2T, xTs_sb[:, 0:h], start=True, stop=False)
            nc.tensor.matmul(agg1, w2T, xTs_sb[:, h : 2 * h], start=True, stop=False)

            # ---------------- main loop ----------------
            staged = set()
            last_i = len(sched) - 1
            for gi, (g, b0, nb) in enumerate(sched):
                if g not in staged:
                    stage_xmsg(g)
                    staged.add(g)

                if gi in pre:
                    et, da, db = pre.pop(gi)
                else:
                    et, da, db = load_granule(gi, g, b0, nb)

                # upcast fp8 dist tiles to f16: channel A on the DVE
                # (single-src copy runs 2x_2p), channel B on the otherwise
                # idle scalar/ACT engine
                daf = pool.tile([P, nb, rows], F16, tag="DAF", bufs=3, name=f"daf{gi}")
                nc.vector.tensor_copy(daf, da)
                dbf = pool.tile([P, nb, rows], F16, tag="DBF", bufs=3, name=f"dbf{gi}")
                nc.scalar.copy(dbf, db)

                # dist products on the DVE via the true TENSOR_TENSOR opcode:
                # unlike TensorScalarPtr (1x only), it has the 2x_1p uop for
                # 16-bit step-1 operands -> ~34us per channel per core
                pa = pool.tile([P, nb, rows], F16, tag="PA", bufs=3, name=f"pa{gi}")
                nc.vector.tensor_tensor(pa, et, daf, mybir.AluOpType.mult)
                pb = pool.tile([P, nb, rows], F16, tag="PB", bufs=3, name=f"pb{gi}")
                nc.vector.tensor_tensor(pb, et, dbf, mybir.AluOpType.mult)

                if gi != last_i:
                    # main GEMM chunks
                    for b in range(nb):
                        ch = g * kb + b0 + b
                        nc.tensor.matmul(
                            agg0, xmsg[:, ch], et[:, b, 0:h], start=False, stop=False
                        )
                        nc.tensor.matmul(
                            agg1, xmsg[:, ch], et[:, b, h : 2 * h], start=False, stop=False
                        )
                    # dist-term chunks, grouped per stationary weight
                    for b in range(nb):
                        nc.tensor.matmul(
                            agg0, wd0, pa[:, b, 0:h], start=False, stop=False
                        )
                        nc.tensor.matmul(
                            agg1, wd0, pa[:, b, h : 2 * h], start=False, stop=False
                        )
                    for b in range(nb):
                        nc.tensor.matmul(
                            agg0, wd1, pb[:, b, 0:h], start=False, stop=False
                        )
                        nc.tensor.matmul(
                            agg1, wd1, pb[:, b, h : 2 * h], start=False, stop=False
                        )
                else:
                    # tail: finish bank 0 entirely first so its copy+store
                    # overlaps bank 1's final matmuls
                    ch = g * kb + b0
                    nc.tensor.matmul(agg0, xmsg[:, ch], et[:, 0, 0:h], start=False, stop=False)
                    nc.tensor.matmul(agg0, wd0, pa[:, 0, 0:h], start=False, stop=False)
                    nc.tensor.matmul(agg0, wd1, pb[:, 0, 0:h], start=False, stop=True)
                    out0 = pool.tile([P, h], F32, tag="osb0")
                    nc.scalar.copy(out0, agg0)
                    # tail stores ride the sync ring: its load triggers are
                    # all done by now, and gpsimd is busy with products
                    nc.sync.dma_start(o_d[:, 0:h], out0)

                    nc.tensor.matmul(agg1, xmsg[:, ch], et[:, 0, h : 2 * h], start=False, stop=False)
                    nc.tensor.matmul(agg1, wd0, pa[:, 0, h : 2 * h], start=False, stop=False)
                    nc.tensor.matmul(agg1, wd1, pb[:, 0, h : 2 * h], start=False, stop=True)
                    out1 = pool.tile([P, h], F32, tag="osb1")
                    nc.scalar.copy(out1, agg1)
                    nc.sync.dma_start(o_d[:, h : 2 * h], out1)

    nc.compile()
    return nc


def _prep_in_maps(inputs, rows, n_cores, kb):
    import ml_dtypes

    f16 = np.float16
    f8 = ml_dtypes.float8_e4m3
    x = np.asarray(inputs["x"], np.float32)
    edges = np.asarray(inputs["edges"], np.float32)
    dist = np.asarray(inputs["distance_matrix"], np.float32)
    w1 = np.asarray(inputs["w1"], np.float32)
    w2 = np.asarray(inputs["w2"], np.float32)
    f = x.shape[1]
    n = edges.shape[1]
    nch = n // P
    ngr = nch // kb

    xT16 = np.ascontiguousarray(x.T.astype(f16))  # [f, n]
    w1xT = np.ascontiguousarray(w1[:, :f].T.astype(f16))
    w2T = np.ascontiguousarray(w2.T.astype(f16))
    w1d = w1[:, f:].astype(f16)  # [F, 2]
    wd0 = np.ascontiguousarray(np.broadcast_to(w1d[:, 0][None, :], (P, f)))
    wd1 = np.ascontiguousarray(np.broadcast_to(w1d[:, 1][None, :], (P, f)))

    def g3(mat, dt):  # [rows, n] f32 -> [ngr, 128, kb, rows], j-major granules
        t = mat.T.astype(dt)  # [n, rows]
        return np.ascontiguousarray(
            t.reshape(ngr, kb, P, rows).transpose(0, 2, 1, 3)
        )

    in_maps = []
    for c in range(n_cores):
        i0, i1 = c * rows, (c + 1) * rows
        in_maps.append(
            {
                "e3": g3(edges[i0:i1], f16),
                "da3": g3(dist[i0:i1, :, 0], f8),
                "db3": g3(dist[i0:i1, :, 1], f8),
                "xT": xT16,
                "xT_self": np.ascontiguousarray(xT16[:, i0:i1]),
                "w1xT": w1xT,
                "w2T": w2T,
                "wd0": wd0,
                "wd1": wd1,
            }
        )
    return in_maps


def _run(inputs, n, rows_per_core, n_cores, kb, trace=False):
    from concourse.bass_utils import run_bass_kernel_spmd

    in_maps = _prep_in_maps(inputs, rows_per_core, n_cores, kb)
    nc = build(n=n, rows=rows_per_core, kb=kb)
    res = run_bass_kernel_spmd(nc, in_maps, core_ids=list(range(n_cores)), trace=trace)

    global LAST_RESULT
    LAST_RESULT = res

    out = np.concatenate([r["outT"].T for r in res.results], axis=0)
    return out


def kernel(**inputs) -> np.ndarray:
    trace = os.environ.get("KERNEL_TRACE", "0") == "1"
    return _run(
        inputs,
        n=N_FULL,
        rows_per_core=N_FULL // N_CORES,
        n_cores=N_CORES,
        kb=KB,
        trace=trace,
    )


# revision 22
# speedup vs baseline: 1.2026x; 1.0371x over previous
"""Trainium2 Bass kernel for a GCN message-passing layer (v2, f16 stream).

Reference computation (per node i):
    out[i] = sum_j edges[i,j] * (w1 @ concat(x[j], dist[i,j])) + w2 @ x[i]
which factors into:
    xmsg = x @ w1x.T                       (w1x = w1[:, :128])
    agg  = edges @ xmsg                    (big GEMM, contraction over j)
    dw   = einsum('ij,ijc->ic', edges, dist)
    out  = agg + dw @ w1d.T + x @ w2.T     (w1d = w1[:, 128:130])

Sharding: rows i (targets) split across 8 NeuronCores; x/w1/w2 replicated.

The kernel is HBM-bound: each core must read its edges slice and both
distance channels once.  v2 streams all three as float16 (48 MB/core vs
96 MB in f32; quantization error ~4e-4 rel L2, far under the 2e-2 gate)
and uploads them pre-transposed so the PE does no on-chip transposes:

  e3/da3/db3: [j, i] granule-major layout, so every DMA descriptor is a
  contiguous multi-KB run per partition and every matmul rhs tile is
  [j-part, i-free] as the PE wants it.

Dist term without a serialized epilogue: the DVE forms p_c = E^T .* D_c^T
per granule (f16), and a rank-1 stationary W_c[j,f] = w1d[f,c] (constant
across j) turns sum_j p_c[j,i] * w1d[f,c] into a regular accumulating
matmul into the same PSUM banks as the main GEMM:
    out^T[f,i] += sum_j W_c[j,f] * p_c[j,i].

Rings: sync HWDGE carries E + D-ch0, scalar HWDGE carries D-ch1, gpsimd
carries the small prologue (weights, x^T pieces) and the output stores.
The last granule is split into single-chunk pieces so the kernel tail
(compute on last-arriving data) is short.
"""

import os

import numpy as np

import concourse.bacc as bacc
import concourse.mybir as mybir
from concourse.tile import TileContext

F32 = mybir.dt.float32
F16 = mybir.dt.float16
F8 = mybir.dt.float8e4
P = 128

# problem dims (hardcoded per contract)
N_FULL = 8192
F_IN = 128
F_OUT = 128
N_CORES = 8
KB = 4  # j-chunks (of 128) per streamed granule

LAST_RESULT = None  # BassKernelResults of the most recent kernel() call


def _sched(nch, kb):
    """Granule schedule: (granule, chunk-offset, n-chunks) tuples.

    The final granule is split into single chunks so the tail compute
    pipeline starts on partial data instead of waiting for the full
    granule.
    """
    ngr = nch // kb
    s = [(g, 0, kb) for g in range(ngr - 1)]
    s += [(ngr - 1, b, 1) for b in range(kb)]
    return s


def build(n=N_FULL, rows=N_FULL // N_CORES, kb=KB):
    f = F_IN
    assert n % P == 0 and rows % 2 == 0
    nch = n // P
    assert nch % kb == 0
    ngr = nch // kb
    h = rows // 2  # output free-dim half, one PSUM bank each
    assert h <= 512
    pw = min(1024, n)  # xT prologue piece width
    npc = n // pw

    nc = bacc.Bacc()
    e3_d = nc.declare_dram_parameter("e3", [ngr, P, kb, rows], F16, isOutput=False)
    # distance channels stream as fp8e4 (8 MB each per core instead of 16);
    # quantization error on the dist term is ~0.5% of its share -> ~4e-3
    # rel L2 overall, still 5x under the gate
    da_d = nc.declare_dram_parameter("da3", [ngr, P, kb, rows], F8, isOutput=False)
    db_d = nc.declare_dram_parameter("db3", [ngr, P, kb, rows], F8, isOutput=False)
    xT_d = nc.declare_dram_parameter("xT", [f, n], F16, isOutput=False)
    xTs_d = nc.declare_dram_parameter("xT_self", [f, rows], F16, isOutput=False)
    w1xT_d = nc.declare_dram_parameter("w1xT", [f, F_OUT], F16, isOutput=False)
    w2T_d = nc.declare_dram_parameter("w2T", [f, F_OUT], F16, isOutput=False)
    wd0_d = nc.declare_dram_parameter("wd0", [P, F_OUT], F16, isOutput=False)
    wd1_d = nc.declare_dram_parameter("wd1", [P, F_OUT], F16, isOutput=False)
    o_d = nc.declare_dram_parameter("outT", [F_OUT, rows], F32, isOutput=True)

    sched = _sched(nch, kb)

    with TileContext(nc) as tc:
        with (
            tc.tile_pool(name="const", bufs=1) as cpool,
            tc.tile_pool(name="stream", bufs=2) as pool,
            tc.tile_pool(name="psum", bufs=1, space="PSUM") as pp,
        ):
            def load_granule(gi, g, b0, nb):
                et = pool.tile([P, nb, rows], F16, tag="E", bufs=4, name=f"et{gi}")
                nc.sync.dma_start(et, e3_d[g, :, b0 : b0 + nb, :])
                da = pool.tile([P, nb, rows], F8, tag="DA8", bufs=4, name=f"da{gi}")
                nc.sync.dma_start(da, da_d[g, :, b0 : b0 + nb, :])
                db = pool.tile([P, nb, rows], F8, tag="DB8", bufs=4, name=f"db{gi}")
                nc.sync.dma_start(db, db_d[g, :, b0 : b0 + nb, :])
                return et, da, db

            # the big streams start at t=0: preload the first two granules
            pre = {}
            for gi in (0, 1):
                pre[gi] = load_granule(gi, *sched[gi])

            # ---------------- prologue (gpsimd ring) ----------------
            w1xT = cpool.tile([f, F_OUT], F16)
            nc.gpsimd.dma_start(w1xT, w1xT_d[:, :])
            w2T = cpool.tile([f, F_OUT], F16)
            nc.gpsimd.dma_start(w2T, w2T_d[:, :])
            wd0 = cpool.tile([P, F_OUT], F16)
            nc.gpsimd.dma_start(wd0, wd0_d[:, :])
            wd1 = cpool.tile([P, F_OUT], F16)
            nc.gpsimd.dma_start(wd1, wd1_d[:, :])
            xTs_sb = cpool.tile([f, rows], F16)
            nc.gpsimd.dma_start(xTs_sb, xTs_d[:, :])
            xTp = []
            for b in range(npc):
                t = cpool.tile([f, pw], F16, name=f"xTp{b}")
                nc.gpsimd.dma_start(t, xT_d[:, b * pw : (b + 1) * pw])
                xTp.append(t)

            # xmsg[j, f] chunks land here (f16, stationary for main GEMM)
            xmsg = cpool.tile([P, nch, f], F16)

            def stage_xmsg(g):
                # one PSUM bank stages the kb chunks of granule g
                xm = pp.tile([P, kb * f], F32, tag="xstage", bufs=2, name=f"xm{g}")
                for r in range(kb):
                    ch = kb * g + r
                    b, off = divmod(ch * P, pw)
                    nc.tensor.matmul(
                        xm[:, r * f : (r + 1) * f],
                        xTp[b][:, off : off + P],
                        w1xT,
                        start=True,
                        stop=True,
                    )
                nc.scalar.copy(xmsg[:, kb * g : kb * (g + 1)], xm)

            # output accumulators: one PSUM bank per output half
            agg0 = pp.tile([P, h], F32, tag="agg0")
            agg1 = pp.tile([P, h], F32, tag="agg1")
            # self-connection term starts the accumulation
            nc.tensor.matmul(agg0, w2T, xTs_sb[:, 0:h], start=True, stop=False)
            nc.tensor.matmul(agg1, w2T, xTs_sb[:, h : 2 * h], start=True, stop=False)

            # ---------------- main loop ----------------
            staged = set()
            last_i = len(sched) - 1
            for gi, (g, b0, nb) in enumerate(sched):
                if g not in staged:
                    stage_xmsg(g)
                    staged.add(g)

                if gi in pre:
                    et, da, db = pre.pop(gi)
                else:
                    et, da, db = load_granule(gi, g, b0, nb)

                # upcast fp8 dist tiles to f16: channel A on the DVE
                # (single-src copy runs 2x_2p), channel B on the otherwise
                # idle scalar/ACT engine
                daf = pool.tile([P, nb, rows], F16, tag="DAF", bufs=2, name=f"daf{gi}")
                if gi % 3 == 0:
                    nc.vector.tensor_copy(daf, da)
                else:
                    nc.scalar.copy(daf, da)
                dbf = pool.tile([P, nb, rows], F16, tag="DBF", bufs=2, name=f"dbf{gi}")
                nc.scalar.copy(dbf, db)

                # dist products on the DVE via the true TENSOR_TENSOR opcode:
                # unlike TensorScalarPtr (1x only), it has the 2x_1p uop for
                # 16-bit step-1 operands -> ~34us per channel per core
                pa = pool.tile([P, nb, rows], F16, tag="PA", bufs=2, name=f"pa{gi}")
                nc.vector.tensor_tensor(pa, et, daf, mybir.AluOpType.mult)
                pb = pool.tile([P, nb, rows], F16, tag="PB", bufs=2, name=f"pb{gi}")
                nc.vector.tensor_tensor(pb, et, dbf, mybir.AluOpType.mult)

                if gi != last_i:
                    # main GEMM chunks
                    for b in range(nb):
                        ch = g * kb + b0 + b
                        nc.tensor.matmul(
                            agg0, xmsg[:, ch], et[:, b, 0:h], start=False, stop=False
                        )
                        nc.tensor.matmul(
                            agg1, xmsg[:, ch], et[:, b, h : 2 * h], start=False, stop=False
                        )
                    # dist-term chunks, grouped per stationary weight
                    for b in range(nb):
                        nc.tensor.matmul(
                            agg0, wd0, pa[:, b, 0:h], start=False, stop=False
                        )
                        nc.tensor.matmul(
                            agg1, wd0, pa[:, b, h : 2 * h], start=False, stop=False
                        )
                    for b in range(nb):
                        nc.tensor.matmul(
                            agg0, wd1, pb[:, b, 0:h], start=False, stop=False
                        )
                        nc.tensor.matmul(
                            agg1, wd1, pb[:, b, h : 2 * h], start=False, stop=False
                        )
                else:
                    # tail: finish bank 0 entirely first so its copy+store
                    # overlaps bank 1's final matmuls
                    ch = g * kb + b0
                    nc.tensor.matmul(agg0, xmsg[:, ch], et[:, 0, 0:h], start=False, stop=False)
                    nc.tensor.matmul(agg0, wd0, pa[:, 0, 0:h], start=False, stop=False)
                    nc.tensor.matmul(agg0, wd1, pb[:, 0, 0:h], start=False, stop=True)
                    out0 = pool.tile([P, h], F32, tag="osb0")
                    nc.scalar.copy(out0, agg0)
                    # tail stores ride the sync ring: its load triggers are
                    # all done by now, and gpsimd is busy with products
                    nc.sync.dma_start(o_d[:, 0:h], out0)

                    nc.tensor.matmul(agg1, xmsg[:, ch], et[:, 0, h : 2 * h], start=False, stop=False)
                    nc.tensor.matmul(agg1, wd0, pa[:, 0, h : 2 * h], start=False, stop=False)
                    nc.tensor.matmul(agg1, wd1, pb[:, 0, h : 2 * h], start=False, stop=True)
                    out1 = pool.tile([P, h], F32, tag="osb1")
                    nc.scalar.copy(out1, agg1)
                    nc.sync.dma_start(o_d[:, h : 2 * h], out1)

    nc.compile()
    return nc


def _prep_in_maps(inputs, rows, n_cores, kb):
    import ml_dtypes

    f16 = np.float16
    f8 = ml_dtypes.float8_e4m3
    x = np.asarray(inputs["x"], np.float32)
    edges = np.asarray(inputs["edges"], np.float32)
    dist = np.asarray(inputs["distance_matrix"], np.float32)
    w1 = np.asarray(inputs["w1"], np.float32)
    w2 = np.asarray(inputs["w2"], np.float32)
    f = x.shape[1]
    n = edges.shape[1]
    nch = n // P
    ngr = nch // kb

    xT16 = np.ascontiguousarray(x.T.astype(f16))  # [f, n]
    w1xT = np.ascontiguousarray(w1[:, :f].T.astype(f16))
    w2T = np.ascontiguousarray(w2.T.astype(f16))
    w1d = w1[:, f:].astype(f16)  # [F, 2]
    wd0 = np.ascontiguousarray(np.broadcast_to(w1d[:, 0][None, :], (P, f)))
    wd1 = np.ascontiguousarray(np.broadcast_to(w1d[:, 1][None, :], (P, f)))

    def g3(mat, dt):  # [rows, n] f32 -> [ngr, 128, kb, rows], j-major granules
        t = mat.T.astype(dt)  # [n, rows]
        return np.ascontiguousarray(
            t.reshape(ngr, kb, P, rows).transpose(0, 2, 1, 3)
        )

    in_maps = []
    for c in range(n_cores):
        i0, i1 = c * rows, (c + 1) * rows
        in_maps.append(
            {
                "e3": g3(edges[i0:i1], f16),
                "da3": g3(dist[i0:i1, :, 0], f8),
                "db3": g3(dist[i0:i1, :, 1], f8),
                "xT": xT16,
                "xT_self": np.ascontiguousarray(xT16[:, i0:i1]),
                "w1xT": w1xT,
                "w2T": w2T,
                "wd0": wd0,
                "wd1": wd1,
            }
        )
    return in_maps


def _run(inputs, n, rows_per_core, n_cores, kb, trace=False):
    from concourse.bass_utils import run_bass_kernel_spmd

    in_maps = _prep_in_maps(inputs, rows_per_core, n_cores, kb)
    nc = build(n=n, rows=rows_per_core, kb=kb)
    res = run_bass_kernel_spmd(nc, in_maps, core_ids=list(range(n_cores)), trace=trace)

    global LAST_RESULT
    LAST_RESULT = res

    out = np.concatenate([r["outT"].T for r in res.results], axis=0)
    return out


def kernel(**inputs) -> np.ndarray:
    trace = os.environ.get("KERNEL_TRACE", "0") == "1"
    return _run(
        inputs,
        n=N_FULL,
        rows_per_core=N_FULL // N_CORES,
        n_cores=N_CORES,
        kb=KB,
        trace=trace,
    )


# revision 23
# speedup vs baseline: 1.2688x; 1.0550x over previous
"""Trainium2 Bass kernel for a GCN message-passing layer (v2, f16 stream).

Reference computation (per node i):
    out[i] = sum_j edges[i,j] * (w1 @ concat(x[j], dist[i,j])) + w2 @ x[i]
which factors into:
    xmsg = x @ w1x.T                       (w1x = w1[:, :128])
    agg  = edges @ xmsg                    (big GEMM, contraction over j)
    dw   = einsum('ij,ijc->ic', edges, dist)
    out  = agg + dw @ w1d.T + x @ w2.T     (w1d = w1[:, 128:130])

Sharding: rows i (targets) split across 8 NeuronCores; x/w1/w2 replicated.

The kernel is HBM-bound: each core must read its edges slice and both
distance channels once.  v2 streams all three as float16 (48 MB/core vs
96 MB in f32; quantization error ~4e-4 rel L2, far under the 2e-2 gate)
and uploads them pre-transposed so the PE does no on-chip transposes:

  e3/da3/db3: [j, i] granule-major layout, so every DMA descriptor is a
  contiguous multi-KB run per partition and every matmul rhs tile is
  [j-part, i-free] as the PE wants it.

Dist term without a serialized epilogue: the DVE forms p_c = E^T .* D_c^T
per granule (f16), and a rank-1 stationary W_c[j,f] = w1d[f,c] (constant
across j) turns sum_j p_c[j,i] * w1d[f,c] into a regular accumulating
matmul into the same PSUM banks as the main GEMM:
    out^T[f,i] += sum_j W_c[j,f] * p_c[j,i].

Rings: sync HWDGE carries E + D-ch0, scalar HWDGE carries D-ch1, gpsimd
carries the small prologue (weights, x^T pieces) and the output stores.
The last granule is split into single-chunk pieces so the kernel tail
(compute on last-arriving data) is short.
"""

import os

import numpy as np

import concourse.bacc as bacc
import concourse.mybir as mybir
from concourse.tile import TileContext

F32 = mybir.dt.float32
F16 = mybir.dt.float16
F8 = mybir.dt.float8e4
P = 128

# problem dims (hardcoded per contract)
N_FULL = 8192
F_IN = 128
F_OUT = 128
N_CORES = 8
KB = 4  # j-chunks (of 128) per streamed granule

LAST_RESULT = None  # BassKernelResults of the most recent kernel() call


def _sched(nch, kb):
    """Granule schedule: (granule, chunk-offset, n-chunks) tuples.

    The final granule is split into single chunks so the tail compute
    pipeline starts on partial data instead of waiting for the full
    granule.
    """
    ngr = nch // kb
    s = [(g, 0, kb) for g in range(ngr - 1)]
    s += [(ngr - 1, b, 1) for b in range(kb)]
    return s


def build(n=N_FULL, rows=N_FULL // N_CORES, kb=KB):
    f = F_IN
    assert n % P == 0 and rows % 2 == 0
    nch = n // P
    assert nch % kb == 0
    ngr = nch // kb
    h = rows // 2  # output free-dim half, one PSUM bank each
    assert h <= 512
    pw = min(1024, n)  # xT prologue piece width
    npc = n // pw

    nc = bacc.Bacc()
    e3_d = nc.declare_dram_parameter("e3", [ngr, P, kb, rows], F16, isOutput=False)
    # distance channels stream as fp8e4 (8 MB each per core instead of 16);
    # quantization error on the dist term is ~0.5% of its share -> ~4e-3
    # rel L2 overall, still 5x under the gate
    da_d = nc.declare_dram_parameter("da3", [ngr, P, kb, rows], F8, isOutput=False)
    db_d = nc.declare_dram_parameter("db3", [ngr, P, kb, rows], F8, isOutput=False)
    xT_d = nc.declare_dram_parameter("xT", [f, n], F16, isOutput=False)
    xTs_d = nc.declare_dram_parameter("xT_self", [f, rows], F16, isOutput=False)
    w1xT_d = nc.declare_dram_parameter("w1xT", [f, F_OUT], F16, isOutput=False)
    w2T_d = nc.declare_dram_parameter("w2T", [f, F_OUT], F16, isOutput=False)
    wd0_d = nc.declare_dram_parameter("wd0", [P, F_OUT], F16, isOutput=False)
    wd1_d = nc.declare_dram_parameter("wd1", [P, F_OUT], F16, isOutput=False)
    o_d = nc.declare_dram_parameter("outT", [F_OUT, rows], F32, isOutput=True)

    sched = _sched(nch, kb)

    with TileContext(nc) as tc:
        with (
            tc.tile_pool(name="const", bufs=1) as cpool,
            tc.tile_pool(name="stream", bufs=2) as pool,
            tc.tile_pool(name="psum", bufs=1, space="PSUM") as pp,
        ):
            def load_granule(gi, g, b0, nb):
                et = pool.tile([P, nb, rows], F16, tag="E", bufs=4, name=f"et{gi}")
                nc.sync.dma_start(et, e3_d[g, :, b0 : b0 + nb, :])
                da = pool.tile([P, nb, rows], F8, tag="DA8", bufs=4, name=f"da{gi}")
                nc.sync.dma_start(da, da_d[g, :, b0 : b0 + nb, :])
                db = pool.tile([P, nb, rows], F8, tag="DB8", bufs=4, name=f"db{gi}")
                nc.sync.dma_start(db, db_d[g, :, b0 : b0 + nb, :])
                return et, da, db

            # the big streams start at t=0: preload the first two granules
            pre = {}
            for gi in (0, 1):
                pre[gi] = load_granule(gi, *sched[gi])

            # ---------------- prologue (gpsimd ring) ----------------
            w1xT = cpool.tile([f, F_OUT], F16)
            nc.gpsimd.dma_start(w1xT, w1xT_d[:, :])
            w2T = cpool.tile([f, F_OUT], F16)
            nc.gpsimd.dma_start(w2T, w2T_d[:, :])
            wd0 = cpool.tile([P, F_OUT], F16)
            nc.gpsimd.dma_start(wd0, wd0_d[:, :])
            wd1 = cpool.tile([P, F_OUT], F16)
            nc.gpsimd.dma_start(wd1, wd1_d[:, :])
            xTs_sb = cpool.tile([f, rows], F16)
            nc.gpsimd.dma_start(xTs_sb, xTs_d[:, :])
            xTp = []
            for b in range(npc):
                t = cpool.tile([f, pw], F16, name=f"xTp{b}")
                nc.gpsimd.dma_start(t, xT_d[:, b * pw : (b + 1) * pw])
                xTp.append(t)

            # xmsg[j, f] chunks land here (f16, stationary for main GEMM)
            xmsg = cpool.tile([P, nch, f], F16)

            def stage_xmsg(g):
                # one PSUM bank stages the kb chunks of granule g
                xm = pp.tile([P, kb * f], F32, tag="xstage", bufs=2, name=f"xm{g}")
                for r in range(kb):
                    ch = kb * g + r
                    b, off = divmod(ch * P, pw)
                    nc.tensor.matmul(
                        xm[:, r * f : (r + 1) * f],
                        xTp[b][:, off : off + P],
                        w1xT,
                        start=True,
                        stop=True,
                    )
                nc.scalar.copy(xmsg[:, kb * g : kb * (g + 1)], xm)

            # output accumulators: one PSUM bank per output half
            agg0 = pp.tile([P, h], F32, tag="agg0")
            agg1 = pp.tile([P, h], F32, tag="agg1")
            # self-connection term starts the accumulation
            nc.tensor.matmul(agg0, w2T, xTs_sb[:, 0:h], start=True, stop=False)
            nc.tensor.matmul(agg1, w2T, xTs_sb[:, h : 2 * h], start=True, stop=False)

            # ---------------- main loop ----------------
            staged = set()
            last_i = len(sched) - 1
            for gi, (g, b0, nb) in enumerate(sched):
                if g not in staged:
                    stage_xmsg(g)
                    staged.add(g)

                if gi in pre:
                    et, da, db = pre.pop(gi)
                else:
                    et, da, db = load_granule(gi, g, b0, nb)

                # upcast fp8 dist tiles to f16: channel A on the DVE
                # (single-src copy runs 2x_2p), channel B on the otherwise
                # idle scalar/ACT engine
                daf = pool.tile([P, nb, rows], F16, tag="DAF", bufs=2, name=f"daf{gi}")
                if gi % 2 == 0:
                    nc.vector.tensor_copy(daf, da)
                else:
                    nc.scalar.copy(daf, da)
                dbf = pool.tile([P, nb, rows], F16, tag="DBF", bufs=2, name=f"dbf{gi}")
                nc.scalar.copy(dbf, db)

                # dist products on the DVE via the true TENSOR_TENSOR opcode:
                # unlike TensorScalarPtr (1x only), it has the 2x_1p uop for
                # 16-bit step-1 operands -> ~34us per channel per core
                pa = pool.tile([P, nb, rows], F16, tag="PA", bufs=2, name=f"pa{gi}")
                nc.vector.tensor_tensor(pa, et, daf, mybir.AluOpType.mult)
                pb = pool.tile([P, nb, rows], F16, tag="PB", bufs=2, name=f"pb{gi}")
                nc.vector.tensor_tensor(pb, et, dbf, mybir.AluOpType.mult)

                if gi != last_i:
                    # main GEMM chunks
                    for b in range(nb):
                        ch = g * kb + b0 + b
                        nc.tensor.matmul(
                            agg0, xmsg[:, ch], et[:, b, 0:h], start=False, stop=False
                        )
                        nc.tensor.matmul(
                            agg1, xmsg[:, ch], et[:, b, h : 2 * h], start=False, stop=False
                        )
                    # dist-term chunks, grouped per stationary weight
                    for b in range(nb):
                        nc.tensor.matmul(
                            agg0, wd0, pa[:, b, 0:h], start=False, stop=False
                        )
                        nc.tensor.matmul(
                            agg1, wd0, pa[:, b, h : 2 * h], start=False, stop=False
                        )
                    for b in range(nb):
                        nc.tensor.matmul(
                            agg0, wd1, pb[:, b, 0:h], start=False, stop=False
                        )
                        nc.tensor.matmul(
                            agg1, wd1, pb[:, b, h : 2 * h], start=False, stop=False
                        )
                else:
                    # tail: finish bank 0 entirely first so its copy+store
                    # overlaps bank 1's final matmuls
                    ch = g * kb + b0
                    nc.tensor.matmul(agg0, xmsg[:, ch], et[:, 0, 0:h], start=False, stop=False)
                    nc.tensor.matmul(agg0, wd0, pa[:, 0, 0:h], start=False, stop=False)
                    nc.tensor.matmul(agg0, wd1, pb[:, 0, 0:h], start=False, stop=True)
                    out0 = pool.tile([P, h], F32, tag="osb0")
                    nc.scalar.copy(out0, agg0)
                    # tail stores ride the sync ring: its load triggers are
                    # all done by now, and gpsimd is busy with products
                    nc.sync.dma_start(o_d[:, 0:h], out0)

                    nc.tensor.matmul(agg1, xmsg[:, ch], et[:, 0, h : 2 * h], start=False, stop=False)
                    nc.tensor.matmul(agg1, wd0, pa[:, 0, h : 2 * h], start=False, stop=False)
                    nc.tensor.matmul(agg1, wd1, pb[:, 0, h : 2 * h], start=False, stop=True)
                    out1 = pool.tile([P, h], F32, tag="osb1")
                    nc.scalar.copy(out1, agg1)
                    nc.sync.dma_start(o_d[:, h : 2 * h], out1)

    nc.compile()
    return nc


def _prep_in_maps(inputs, rows, n_cores, kb):
    import ml_dtypes

    f16 = np.float16
    f8 = ml_dtypes.float8_e4m3
    x = np.asarray(inputs["x"], np.float32)
    edges = np.asarray(inputs["edges"], np.float32)
    dist = np.asarray(inputs["distance_matrix"], np.float32)
    w1 = np.asarray(inputs["w1"], np.float32)
    w2 = np.asarray(inputs["w2"], np.float32)
    f = x.shape[1]
    n = edges.shape[1]
    nch = n // P
    ngr = nch // kb

    xT16 = np.ascontiguousarray(x.T.astype(f16))  # [f, n]
    w1xT = np.ascontiguousarray(w1[:, :f].T.astype(f16))
    w2T = np.ascontiguousarray(w2.T.astype(f16))
    w1d = w1[:, f:].astype(f16)  # [F, 2]
    wd0 = np.ascontiguousarray(np.broadcast_to(w1d[:, 0][None, :], (P, f)))
    wd1 = np.ascontiguousarray(np.broadcast_to(w1d[:, 1][None, :], (P, f)))

    def g3(mat, dt):  # [rows, n] f32 -> [ngr, 128, kb, rows], j-major granules
        t = mat.T.astype(dt)  # [n, rows]
        return np.ascontiguousarray(
            t.reshape(ngr, kb, P, rows).transpose(0, 2, 1, 3)
        )

    in_maps = []
    for c in range(n_cores):
        i0, i1 = c * rows, (c + 1) * rows
        in_maps.append(
            {
                "e3": g3(edges[i0:i1], f16),
                "da3": g3(dist[i0:i1, :, 0], f8),
                "db3": g3(dist[i0:i1, :, 1], f8),
                "xT": xT16,
                "xT_self": np.ascontiguousarray(xT16[:, i0:i1]),
                "w1xT": w1xT,
                "w2T": w2T,
                "wd0": wd0,
                "wd1": wd1,
            }
        )
    return in_maps


def _run(inputs, n, rows_per_core, n_cores, kb, trace=False):
    from concourse.bass_utils import run_bass_kernel_spmd

    in_maps = _prep_in_maps(inputs, rows_per_core, n_cores, kb)
    nc = build(n=n, rows=rows_per_core, kb=kb)
    res = run_bass_kernel_spmd(nc, in_maps, core_ids=list(range(n_cores)), trace=trace)

    global LAST_RESULT
    LAST_RESULT = res

    out = np.concatenate([r["outT"].T for r in res.results], axis=0)
    return out


def kernel(**inputs) -> np.ndarray:
    trace = os.environ.get("KERNEL_TRACE", "0") == "1"
    return _run(
        inputs,
        n=N_FULL,
        rows_per_core=N_FULL // N_CORES,
        n_cores=N_CORES,
        kb=KB,
        trace=trace,
    )
